# revision 70
# baseline (speedup 1.0000x reference)
"""GCNConv layer on 8 Trainium2 NeuronCores (Bass/Tile).

Strategy (graph/data parallel, edges partitioned by destination):
  out = relu( D^-1/2 (A+I) D^-1/2 (x W) + b ) + x
      = relu( (dinv_d * (sum_{e->d} dinv_s x_s + dinv_d x_d)) @ W + b ) + x
(using linearity: the W matmul is applied after aggregation).

Each core owns N/8 destination nodes. Per core:
  - source nodes are split into 4 chunks of N/4 rows so gather indices fit
    int16 (dma_gather requirement)
  - per (chunk c, dst-half h): destinations ordered by in-degree from chunk c
    (descending), so the k-th incoming edge of every dst forms a *prefix* of
    the ordering (ELL layout).  Consecutive passes are fused into dma_gathers
    of up to ~8k rows; each fused gather is scaled by dinv_src (DVE, using a
    host-shipped weight blob) and accumulated into the phase accumulator with
    per-pass offset views (pass boundaries are 128-row aligned).
  - gathers are spread round-robin over the 4 SWDGE queues for completion-sem
    lane isolation. NOTE: descriptor generation itself is strictly serial on
    the Pool engine (~3.3ns/row when unstalled) and is the kernel's critical
    path; the 6-deep gather-buffer pool keeps the Q7 from stalling on DVE
    consumption (bufs=4 -> 6 alone was a 1.43x end-to-end win).
  - per-phase accumulator is scaled by dinv_d (rank order); chunks 1..3 are
    written *densely* to per-chunk HBM tables and re-gathered per 1024-row
    group, while chunk 0's accumulator STAYS IN SBUF (bf16): the epilogue
    runs in chunk-0 rank space, so chunk 0 needs no merge re-gather (-12.5k
    Q7 rows) and the host un-permutes the output rows afterwards.
  - final: sum acc0 (SBUF) + 3 gathered tables + self-loop term dinv_d^2 x_d,
    transpose via PE, matmul with W, fused bias+relu on ACT, transpose back,
    add residual x (bf16), store; host scatters rows back to natural order.

Edge sets are padded with weight-0 fake edges so all 8 cores run the exact
same static program (SPMD) with per-core data only.
"""

import sys
import types

sys.path.insert(0, "/opt/trn_rl_repo")

import ml_dtypes
import numpy as np

DIM = 64
N_CORES = 8
N_CHUNKS = 4
N_HALVES = 2
N_QUEUES = 4
P = 128
GB = 8       # dst blocks per final-phase group
GBM = 8      # dst blocks per merge-gather group
FUSE_CAP = 8192  # max slots per fused gather (single passes may exceed)
SINGLE_PACKET = False


def _install_ntff_hook():
    if "antenv.axon_hooks" in sys.modules:
        return
    try:
        sys.path.insert(0, "/root/.axon_site")
        from trn_agent_boot.trn_boot import _ntff_profile_via_ctypes

        hook = _ntff_profile_via_ctypes("/opt/axon/libaxon_pjrt.so")
    except Exception:
        hook = None
    mod = types.ModuleType("antenv.axon_hooks")
    mod.get_axon_ntff_profile_hook = lambda: hook
    mod.set_axon_ntff_profile_hook = lambda h: None
    sys.modules["antenv.axon_hooks"] = mod


class Plan:
    def __init__(self, n_nodes, n_cores, n_chunks, n_halves):
        assert n_nodes % n_cores == 0
        assert n_nodes % n_chunks == 0
        self.N = n_nodes
        self.n_cores = n_cores
        self.n_chunks = n_chunks
        self.n_halves = n_halves
        self.SHARD = n_nodes // n_cores
        self.CH = n_nodes // n_chunks
        assert self.CH <= 32767, "chunk must fit int16 index"
        assert self.SHARD % n_halves == 0
        self.HALF = self.SHARD // n_halves
        self.SHB = -(-self.SHARD // P)
        self.ACCB = -(-self.HALF // P)
        self.ACC_SLOTS = self.ACCB * P
        self.n_phases = n_chunks * n_halves
        self.n_groups = -(-self.SHB // GBM)
        self.pass_sizes = None
        self.g16_off = None
        self.g128_off = None
        self.fuse = None     # [phase] -> list of (k_start, k_end, slots)
        self.GCOLS = 0
        self.WCOLS = 0
        self.group_sizes = [
            min(GBM, self.SHB - g * GBM) * P for g in range(self.n_groups)
        ]
        # chunk 0's accumulator stays in SBUF (epilogue runs in chunk-0 rank
        # space); only chunks 1..n_chunks-1 need merge re-gathers.
        self.n_mg = n_chunks - 1
        self.MG_COLS = sum(s // 16 for s in self.group_sizes) * self.n_mg
        self.out_perm = None   # [core] -> global row idx per valid slot
        self.valid_slots = None

    def mg_off(self, g, cm):
        o = 0
        for gg in range(g):
            o += (self.group_sizes[gg] // 16) * self.n_mg
        return o + (self.group_sizes[g] // 16) * cm


def _rep16(vals_i16, n):
    a = np.asarray(vals_i16, dtype=np.int16).reshape(n // 16, 16).T
    return np.tile(a, (8, 1))


def preprocess(x, edge_index, W, b):
    x = np.ascontiguousarray(np.asarray(x, dtype=np.float32))
    N = x.shape[0]
    plan = Plan(N, N_CORES, N_CHUNKS, N_HALVES)
    src = np.asarray(edge_index[0], dtype=np.int64)
    dst = np.asarray(edge_index[1], dtype=np.int64)
    deg = np.bincount(dst, minlength=N).astype(np.float64) + 1.0
    dinv = (1.0 / np.sqrt(deg)).astype(np.float32)

    SHARD, CH, HALF = plan.SHARD, plan.CH, plan.HALF
    NPH = plan.n_phases

    core_of = dst // SHARD
    per_core = []
    for i in range(N_CORES):
        m = core_of == i
        s_i = src[m]
        d_i = dst[m] - i * SHARD
        c_i = s_i // CH
        h_i = d_i // HALF
        phases = []
        for c in range(N_CHUNKS):
            for h in range(N_HALVES):
                mm = (c_i == c) & (h_i == h)
                s = s_i[mm]
                d = d_i[mm] - h * HALF
                deg_ch = np.bincount(d, minlength=HALF)
                order = np.argsort(-deg_ch, kind="stable")
                rank = np.empty(HALF, dtype=np.int64)
                rank[order] = np.arange(HALF)
                perm = np.argsort(rank[d], kind="stable")
                s_sorted = s[perm]
                counts = deg_ch[order]
                cum = np.concatenate([[0], np.cumsum(counts)])
                K = int(counts[0]) if len(s) else 0
                passes = []
                for k in range(K):
                    L = int(np.searchsorted(-counts, -k, side="left"))
                    passes.append(s_sorted[cum[:L] + k])
                phases.append({"passes": passes, "rank": rank})
        per_core.append(phases)

    pass_sizes = []
    for ph in range(NPH):
        K = max(len(per_core[i][ph]["passes"]) for i in range(N_CORES))
        sizes = []
        for k in range(K):
            L = max(
                len(per_core[i][ph]["passes"][k])
                if k < len(per_core[i][ph]["passes"])
                else 0
                for i in range(N_CORES)
            )
            sizes.append(-(-L // P) * P)
        pass_sizes.append(sizes)
    plan.pass_sizes = pass_sizes

    # fused gather grouping (never split a pass; an oversize single pass
    # forms its own group)
    cap = FUSE_CAP
    fuse = []
    for ph in range(NPH):
        groups = []
        k0, tot = 0, 0
        for k, n in enumerate(pass_sizes[ph]):
            if tot and tot + n > cap:
                groups.append((k0, k, tot))
                k0, tot = k, 0
            tot += n
        if tot:
            groups.append((k0, len(pass_sizes[ph]), tot))
        fuse.append(groups)
    plan.fuse = fuse

    g16_off, g128_off = [], []
    o16 = o128 = 0
    for ph in range(NPH):
        offs16, offs128 = [], []
        for n in pass_sizes[ph]:
            offs16.append(o16)
            offs128.append(o128)
            o16 += n // 16
            o128 += n // P
        g16_off.append(offs16)
        g128_off.append(offs128)
    plan.g16_off, plan.g128_off = g16_off, g128_off
    plan.GCOLS = max(o16, 16)
    plan.WCOLS = max(o128, 1)

    in_maps = []
    out_perm = []
    W = np.ascontiguousarray(np.asarray(W, dtype=np.float32))
    b = np.ascontiguousarray(np.asarray(b, dtype=np.float32).reshape(DIM, 1))
    for i in range(N_CORES):
        gidx = np.zeros((P, plan.GCOLS), dtype=np.int16)
        gwgt = np.zeros((P, plan.WCOLS), dtype=np.float32)
        for ph in range(NPH):
            c = ph // N_HALVES
            pdata = per_core[i][ph]
            for k, n in enumerate(pass_sizes[ph]):
                s_pass = (
                    pdata["passes"][k]
                    if k < len(pdata["passes"])
                    else np.empty(0, np.int64)
                )
                L = len(s_pass)
                iv = np.zeros(n, dtype=np.int16)
                wv = np.zeros(n, dtype=np.float32)
                iv[:L] = (s_pass - c * CH).astype(np.int16)
                wv[:L] = dinv[s_pass]
                gidx[:, plan.g16_off[ph][k] : plan.g16_off[ph][k] + n // 16] = (
                    _rep16(iv, n)
                )
                gwgt[:, plan.g128_off[ph][k] : plan.g128_off[ph][k] + n // P] = (
                    wv.reshape(n // P, P).T
                )
        dinvr = np.zeros((P, NPH * plan.ACCB), dtype=np.float32)
        for ph in range(NPH):
            h = ph % N_HALVES
            rank = per_core[i][ph]["rank"]
            dv = np.zeros(plan.ACC_SLOTS, dtype=np.float32)
            order = np.empty(HALF, dtype=np.int64)
            order[rank] = np.arange(HALF)
            dv[:HALF] = dinv[i * SHARD + h * HALF + order]
            dinvr[:, ph * plan.ACCB : (ph + 1) * plan.ACCB] = dv.reshape(
                plan.ACCB, P
            ).T
        # chunk-0 rank-space slot maps: the epilogue runs in chunk-0 rank
        # order; slot s -> (half h = s // ACC_SLOTS, rank r = s % ACC_SLOTS).
        order0 = []
        for h in range(N_HALVES):
            rank0 = per_core[i][h]["rank"]
            o_ = np.empty(HALF, dtype=np.int64)
            o_[rank0] = np.arange(HALF)
            order0.append(o_)
        order0_cat = np.concatenate(order0)

        mgidx = np.zeros((P, plan.MG_COLS), dtype=np.int16)
        for g in range(plan.n_groups):
            gsz = plan.group_sizes[g]
            s = np.arange(g * GBM * P, g * GBM * P + gsz)
            hs = s // plan.ACC_SLOTS
            rs = s % plan.ACC_SLOTS
            ok = rs < HALF
            for c in range(1, N_CHUNKS):
                iv = np.empty(gsz, dtype=np.int16)
                for h in range(N_HALVES):
                    rank_ch = per_core[i][c * N_HALVES + h]["rank"]
                    mh = ok & (hs == h)
                    iv[mh] = (
                        h * plan.ACC_SLOTS + rank_ch[order0[h][rs[mh]]]
                    ).astype(np.int16)
                iv[~ok] = ((hs[~ok] + 1) * plan.ACC_SLOTS - 1).astype(np.int16)
                o = plan.mg_off(g, c - 1)
                mgidx[:, o : o + gsz // 16] = _rep16(iv, gsz)

        s = np.arange(plan.SHB * P)
        hs = s // plan.ACC_SLOTS
        rs = s % plan.ACC_SLOTS
        ok = rs < HALF
        d_loc = np.zeros(plan.SHB * P, dtype=np.int64)
        d_loc[ok] = hs[ok] * HALF + order0_cat[hs[ok] * HALF + rs[ok]]
        dv = np.zeros((plan.SHB * P,), dtype=np.float32)
        dv[ok] = dinv[i * SHARD + d_loc[ok]] ** 2
        dinvsq = np.ascontiguousarray(dv.reshape(plan.SHB, P).T)
        xsl = np.zeros((plan.SHB * P, DIM), dtype=np.float32)
        xsl[ok] = x[i * SHARD + d_loc[ok]]
        xsl = xsl.reshape(plan.SHB, P, DIM).transpose(1, 0, 2).reshape(
            plan.SHB * P, DIM
        )
        xsl = np.ascontiguousarray(xsl.astype(ml_dtypes.bfloat16))
        out_perm.append(i * SHARD + d_loc[ok])
        plan.valid_slots = ok
        in_maps.append(
            {
                "x": x,
                "xsl": xsl,
                "w": W,
                "bias": b,
                "dinvsq": dinvsq,
                "dinvr": dinvr,
                "gidx": gidx,
                "gwgt": gwgt,
                "mgidx": mgidx,
            }
        )
    plan.out_perm = out_perm
    return plan, in_maps


_QPATCHED = [False]


def _patch_queue_aware_dma_lanes():
    """Partition the 8 DMASW completion-sem lanes so SWDGE queue q owns
    lanes {2q, 2q+1} (Tile's round-robin ignores queue_num; queues sharing a
    lane can complete out of order and release waiters early)."""
    if _QPATCHED[0]:
        return
    _QPATCHED[0] = True
    from concourse import tile_sem_assignment as tsa
    from concourse import bass_isa, mybir

    orig = tsa.TileClockTick._assign_tick

    def qaware(self, inst):
        if (
            isinstance(inst, tsa.DMAInst)
            and inst.engine == mybir.EngineType.Pool
            and not isinstance(inst, bass_isa.UserSyncedRemoteDMADescs)
        ):
            qn = getattr(inst, "queue_num", 0) or 0
            tog = getattr(self, "_q_toggle", None)
            if tog is None:
                tog = self._q_toggle = {}
            t = tog.get(qn, 0)
            tog[qn] = t ^ 1
            self.next_sw_dma_idx = 2 * qn + t
        return orig(self, inst)

    tsa.TileClockTick._assign_tick = qaware


def build_program(plan):
    from concourse import bacc, mybir
    import concourse.tile as tile
    from concourse.masks import make_identity
    from concourse.tile import add_dep_helper

    _patch_queue_aware_dma_lanes()

    N = plan.N
    SHARD, CH, HALF = plan.SHARD, plan.CH, plan.HALF
    SHB, ACCB = plan.SHB, plan.ACCB
    NPH = plan.n_phases
    FB = SHARD // P
    REM = SHARD - FB * P
    f32 = mybir.dt.float32
    bf16 = mybir.dt.bfloat16
    i16 = mybir.dt.int16
    mult = mybir.AluOpType.mult
    add = mybir.AluOpType.add

    nc = bacc.Bacc("TRN2", target_bir_lowering=False, num_swdge_queues=N_QUEUES)
    x_d = nc.dram_tensor("x", [N, DIM], f32, kind="ExternalInput")
    xsl_d = nc.dram_tensor("xsl", [SHB * P, DIM], bf16, kind="ExternalInput")
    w_d = nc.dram_tensor("w", [DIM, DIM], f32, kind="ExternalInput")
    b_d = nc.dram_tensor("bias", [DIM, 1], f32, kind="ExternalInput")
    dinvsq_d = nc.dram_tensor("dinvsq", [P, SHB], f32, kind="ExternalInput")
    dinvr_d = nc.dram_tensor("dinvr", [P, NPH * ACCB], f32, kind="ExternalInput")
    gidx_d = nc.dram_tensor("gidx", [P, plan.GCOLS], i16, kind="ExternalInput")
    gwgt_d = nc.dram_tensor("gwgt", [P, plan.WCOLS], f32, kind="ExternalInput")
    mgidx_d = nc.dram_tensor("mgidx", [P, plan.MG_COLS], i16, kind="ExternalInput")
    accd = [
        nc.dram_tensor(f"accd{c}", [N_HALVES * plan.ACC_SLOTS, DIM], f32)
        for c in range(1, N_CHUNKS)
    ]
    out_d = nc.dram_tensor("out", [SHB * P, DIM], f32, kind="ExternalOutput")

    max_big = max((g[2] for groups in plan.fuse for g in groups), default=P)
    qctr = [0]

    def next_q():
        q = qctr[0] % N_QUEUES
        qctr[0] += 1
        return q

    with tile.TileContext(nc) as tc:
        with (
            tc.tile_pool(name="const", bufs=1) as constp,
            tc.tile_pool(name="io", bufs=3) as iop,
            tc.tile_pool(name="gbuf", bufs=6) as gbufp,
            tc.tile_pool(name="acc0", bufs=1) as acc0p,
            tc.tile_pool(name="accp", bufs=2) as accp,
            tc.tile_pool(name="fin", bufs=2) as finp,
            tc.tile_pool(name="psum", bufs=2, space="PSUM") as psump,
            tc.tile_pool(name="psum1", bufs=1, space="PSUM") as psum1p,
        ):
            ident = constp.tile([P, P], f32)
            make_identity(nc, ident[:])
            w_t = constp.tile([DIM, DIM], f32)
            nc.sync.dma_start(out=w_t[:], in_=w_d[:])
            b_t = constp.tile([DIM, 1], f32)
            nc.sync.dma_start(out=b_t[:], in_=b_d[:])
            dinvsq_t = constp.tile([P, SHB], f32)
            nc.sync.dma_start(out=dinvsq_t[:], in_=dinvsq_d[:])
            dinvr_t = constp.tile([P, NPH * ACCB], f32)
            nc.sync.dma_start(out=dinvr_t[:], in_=dinvr_d[:])
            mgidx_t = constp.tile([P, plan.MG_COLS], i16)
            nc.sync.dma_start(out=mgidx_t[:], in_=mgidx_d[:])

            xs_t = constp.tile([P, SHB * DIM], bf16)
            nc.scalar.dma_start(
                out=xs_t[:].rearrange("p (bb d) -> p bb d", d=DIM),
                in_=xsl_d[:, :].rearrange("(p bb) d -> p bb d", p=P),
            )

            accd_writes = []
            acc0_tiles = [None] * N_HALVES
            for ph in range(NPH):
                c = ph // N_HALVES
                h = ph % N_HALVES
                sizes = plan.pass_sizes[ph]
                if sizes:
                    gcols = plan.g16_off[ph][-1] + sizes[-1] // 16 - plan.g16_off[ph][0]
                    wcols = plan.g128_off[ph][-1] + sizes[-1] // P - plan.g128_off[ph][0]
                    gidx_t = iop.tile([P, gcols], i16, tag="gidx")
                    nc.sync.dma_start(
                        out=gidx_t[:],
                        in_=gidx_d[
                            :, plan.g16_off[ph][0] : plan.g16_off[ph][0] + gcols
                        ],
                    )
                    gwgt_t = iop.tile([P, wcols], f32, tag="gwgt")
                    nc.sync.dma_start(
                        out=gwgt_t[:],
                        in_=gwgt_d[
                            :, plan.g128_off[ph][0] : plan.g128_off[ph][0] + wcols
                        ],
                    )
                if c == 0:
                    acc_t = acc0p.tile([P, ACCB * DIM], bf16, tag=f"h{h}")
                    acc0_tiles[h] = acc_t
                else:
                    acc_t = accp.tile([P, ACCB * DIM], f32, tag="acc")
                n0blk = (sizes[0] // P) if sizes else 0
                if n0blk < ACCB:
                    nc.vector.memset(acc_t[:, n0blk * DIM :], 0.0)
                for gi, (k0, k1, slots) in enumerate(plan.fuse[ph]):
                    nblk = slots // P
                    buf = gbufp.tile([P, (max_big // P) * DIM], f32, tag="gb")
                    o16 = plan.g16_off[ph][k0] - plan.g16_off[ph][0]
                    o128 = plan.g128_off[ph][k0] - plan.g128_off[ph][0]
                    nc.gpsimd.dma_gather(
                        out_ap=buf[:, : nblk * DIM].rearrange(
                            "p (j d) -> p j d", d=DIM
                        ),
                        in_ap=x_d[c * CH : (c + 1) * CH, :],
                        idxs_ap=gidx_t[:, o16 : o16 + slots // 16],
                        num_idxs=slots,
                        num_idxs_reg=slots,
                        elem_size=DIM,
                        single_packet=SINGLE_PACKET,
                        queue_num=next_q(),
                    )
                    nc.vector.tensor_tensor(
                        out=buf[:, : nblk * DIM].rearrange("p (j d) -> p j d", d=DIM),
                        in0=buf[:, : nblk * DIM].rearrange("p (j d) -> p j d", d=DIM),
                        in1=gwgt_t[:, o128 : o128 + nblk].to_broadcast(
                            [P, nblk, DIM]
                        ),
                        op=mult,
                    )
                    boff = 0
                    for k in range(k0, k1):
                        nb = sizes[k] // P
                        if gi == 0 and k == k0:
                            nc.vector.tensor_copy(
                                out=acc_t[:, : nb * DIM],
                                in_=buf[:, boff * DIM : (boff + nb) * DIM],
                            )
                        else:
                            nc.vector.tensor_tensor(
                                out=acc_t[:, : nb * DIM],
                                in0=acc_t[:, : nb * DIM],
                                in1=buf[:, boff * DIM : (boff + nb) * DIM],
                                op=add,
                            )
                        boff += nb
                nc.vector.tensor_tensor(
                    out=acc_t[:].rearrange("p (j d) -> p j d", d=DIM),
                    in0=acc_t[:].rearrange("p (j d) -> p j d", d=DIM),
                    in1=dinvr_t[:, ph * ACCB : (ph + 1) * ACCB].to_broadcast(
                        [P, ACCB, DIM]
                    ),
                    op=mult,
                )
                if c > 0:
                    winst = nc.scalar.dma_start(
                        out=accd[c - 1][
                            h * plan.ACC_SLOTS : (h + 1) * plan.ACC_SLOTS, :
                        ].rearrange("(j p) d -> p j d", p=P),
                        in_=acc_t[:].rearrange("p (j d) -> p j d", d=DIM),
                    )
                    accd_writes.append((c, winst))

            for g in range(plan.n_groups):
                gsz = plan.group_sizes[g]
                blks = gsz // P
                mg = []
                for c in range(1, N_CHUNKS):
                    mb = finp.tile([P, GBM * DIM], f32, tag=f"mg{c}")
                    o = plan.mg_off(g, c - 1)
                    ginst = nc.gpsimd.dma_gather(
                        out_ap=mb[:, : blks * DIM].rearrange(
                            "p (j d) -> p j d", d=DIM
                        ),
                        in_ap=accd[c - 1][:, :],
                        idxs_ap=mgidx_t[:, o : o + gsz // 16],
                        num_idxs=gsz,
                        num_idxs_reg=gsz,
                        elem_size=DIM,
                        single_packet=SINGLE_PACKET,
                        queue_num=next_q(),
                    )
                    for cc, wi in accd_writes:
                        if cc == c:
                            add_dep_helper(
                                ginst.ins, wi.ins, reason="accd write before merge"
                            )
                    mg.append(mb)
                # epilogue in windows of GB blocks within this merge group
                for w0 in range(0, blks, GB):
                    wb = min(GB, blks - w0)
                    gb0 = g * GBM + w0  # absolute block offset
                    ag = finp.tile([P, GB * DIM], f32, tag="ag")
                    nc.vector.tensor_tensor(
                        out=ag[:, : wb * DIM].rearrange(
                            "p (bb d) -> p bb d", d=DIM
                        ),
                        in0=xs_t[:, gb0 * DIM : (gb0 + wb) * DIM].rearrange(
                            "p (bb d) -> p bb d", d=DIM
                        ),
                        in1=dinvsq_t[:, gb0 : gb0 + wb].to_broadcast(
                            [P, wb, DIM]
                        ),
                        op=mult,
                    )
                    for h in range(N_HALVES):
                        b0 = max(gb0, h * ACCB)
                        b1 = min(gb0 + wb, (h + 1) * ACCB)
                        if b0 < b1:
                            nc.vector.tensor_tensor(
                                out=ag[:, (b0 - gb0) * DIM : (b1 - gb0) * DIM],
                                in0=ag[:, (b0 - gb0) * DIM : (b1 - gb0) * DIM],
                                in1=acc0_tiles[h][
                                    :,
                                    (b0 - h * ACCB) * DIM : (b1 - h * ACCB)
                                    * DIM,
                                ],
                                op=add,
                            )
                    for mb in mg:
                        nc.vector.tensor_tensor(
                            out=ag[:, : wb * DIM],
                            in0=ag[:, : wb * DIM],
                            in1=mb[:, w0 * DIM : (w0 + wb) * DIM],
                            op=add,
                        )
                    pt = psum1p.tile([DIM, GB * P], f32, tag="pt")
                    for bb in range(wb):
                        nc.tensor.transpose(
                            out=pt[:, bb * P : (bb + 1) * P],
                            in_=ag[:, bb * DIM : (bb + 1) * DIM],
                            identity=ident[:],
                        )
                    at = finp.tile([DIM, GB * P], f32, tag="at")
                    nc.scalar.activation(
                        out=at[:, : wb * P],
                        in_=pt[:, : wb * P],
                        func=mybir.ActivationFunctionType.Copy,
                    )
                    pz = psum1p.tile([DIM, GB * P], f32, tag="pz")
                    for mo in range(0, wb * P, 512):
                        mw = min(512, wb * P - mo)
                        nc.tensor.matmul(
                            out=pz[:, mo : mo + mw],
                            lhsT=w_t[:],
                            rhs=at[:, mo : mo + mw],
                            start=True,
                            stop=True,
                        )
                    zr = finp.tile([DIM, GB * P], f32, tag="zr")
                    nc.scalar.activation(
                        out=zr[:, : wb * P],
                        in_=pz[:, : wb * P],
                        func=mybir.ActivationFunctionType.Relu,
                        bias=b_t[:],
                    )
                    po = psump.tile([P, GB * DIM], f32, tag="po")
                    for bb in range(wb):
                        nc.tensor.transpose(
                            out=po[:, bb * DIM : (bb + 1) * DIM],
                            in_=zr[:, bb * P : (bb + 1) * P],
                            identity=ident[:DIM, :DIM],
                        )
                    ot = finp.tile([P, GB * DIM], f32, tag="ot")
                    nc.vector.tensor_tensor(
                        out=ot[:, : wb * DIM],
                        in0=po[:, : wb * DIM],
                        in1=xs_t[:, gb0 * DIM : (gb0 + wb) * DIM],
                        op=add,
                    )
                    row0 = gb0 * P
                    nc.sync.dma_start(
                        out=out_d[row0 : row0 + wb * P, :].rearrange(
                            "(bb p) d -> p bb d", p=P
                        ),
                        in_=ot[:, : wb * DIM].rearrange(
                            "p (bb d) -> p bb d", d=DIM
                        ),
                    )

    nc.compile()
    return nc


def run(plan, nc, in_maps, trace=False, tmpdir=None):
    _install_ntff_hook()
    from concourse.bass_utils import run_bass_kernel_spmd

    res = run_bass_kernel_spmd(
        nc,
        in_maps,
        core_ids=list(range(plan.n_cores)),
        trace=trace,
        tmpdir=tmpdir,
    )
    out = np.empty((plan.N, DIM), dtype=np.float32)
    ok = plan.valid_slots
    for i in range(plan.n_cores):
        out[plan.out_perm[i]] = np.asarray(
            res.results[i]["out"], dtype=np.float32
        )[ok]
    return out, res


_CACHE = {}


def kernel(x, edge_index, W, b):
    plan, in_maps = preprocess(x, edge_index, W, b)
    sig = tuple(tuple(s) for s in plan.pass_sizes)
    ent = _CACHE.get("prog")
    if ent is None or ent[0] != sig:
        nc = build_program(plan)
        _CACHE["prog"] = (sig, nc)
    nc = _CACHE["prog"][1]
    out, _ = run(plan, nc, in_maps)
    return out

